# revision 14
# baseline (speedup 1.0000x reference)
"""Trainium2 Bass kernel: AttentionRNN (GRUCell step + Bahdanau attention).

Strategy (8 NeuronCores, data-parallel over batch):
  - Each core handles NB=8 batches. No collectives; host scatters/gathers.
  - annotations are staged in HBM in BOTH layouts as bf16:
      annT [b, h, t]  (H on partitions)  -> rhs of the pa = ann @ Wa.T matmul
      annN [b, t, h]  (T on partitions)  -> rhs of the ctx = align @ ann matmul
    Total HBM read stays ~16 MiB/core (same as fp32 single layout), but no
    on-chip transposes of the big tensor are needed.
  - GRU/pq weights are host-pretransposed; all biases are baked in (ones-row
    augmentation for the GRU, pq-bias folded into the tanh activation bias).
  - Per-batch score rows are packed 4-to-a-PSUM-tile with column-tiled
    matmuls (tile_position=(0,32*j)) so softmax/exp run on multi-partition
    tiles.
"""

import os
import numpy as np
import ml_dtypes

B, T_FULL, H = 64, 2048, 256
NCORES = 8
NB = B // NCORES          # batches per core
KH = H // 128             # contraction chunks over H
GROUP = 4                 # batches per psum col-tiling group
NG = NB // GROUP
BF16 = ml_dtypes.bfloat16

_PROGRAM_CACHE = {}


def _build_program(T):
    import concourse.bass as bass
    import concourse.mybir as mybir
    import concourse.tile as tile
    from concourse import bacc
    from concourse.masks import make_identity
    from contextlib import ExitStack

    dt = mybir.dt
    f32 = dt.float32
    bf = dt.bfloat16
    AF = mybir.ActivationFunctionType
    f32r = dt.float32r
    AX = mybir.AxisListType

    NTC = T // 128        # token chunks of 128
    PA_W = min(1024, T)   # pa psum tile width (tokens)
    NTH = T // PA_W       # pa psum tiles per (b, m)
    NQ = PA_W // 512      # 512-wide matmuls per pa psum tile
    NS = T // 512         # score sub-blocks of 512

    nc = bacc.Bacc("TRN2", target_bir_lowering=False, debug=False)

    # ---- DRAM I/O ----
    annT = nc.dram_tensor("annT", [NB * KH, 128, T], bf, kind="ExternalInput").ap()
    annN = nc.dram_tensor("annN", [NB, 128, NTC, H], bf, kind="ExternalInput").ap()
    WaT_d = nc.dram_tensor("WaT", [KH, 128, H], bf, kind="ExternalInput").ap()
    v_d = nc.dram_tensor("vP", [128, KH * 32], bf, kind="ExternalInput").ap()
    xTa_d = nc.dram_tensor("xTa", [5, 128, NB], f32, kind="ExternalInput").ap()
    WihTa_d = nc.dram_tensor("WihTa", [5, 128, 3 * H], f32, kind="ExternalInput").ap()
    rsT_d = nc.dram_tensor("rsT", [KH + 1, 128, NB], f32, kind="ExternalInput").ap()
    WhhT_d = nc.dram_tensor("WhhT", [KH + 1, 128, 3 * H], f32, kind="ExternalInput").ap()
    WqT_d = nc.dram_tensor("WqT", [KH, 128, H], f32, kind="ExternalInput").ap()
    bqa_d = nc.dram_tensor("bqa", [KH, 128, 1], f32, kind="ExternalInput").ap()
    rsN_d = nc.dram_tensor("rsN", [NB, H], f32, kind="ExternalInput").ap()

    rnn_d = nc.dram_tensor("rnn_out", [NB, H], f32, kind="ExternalOutput").ap()
    align_d = nc.dram_tensor("align", [NB, T], f32, kind="ExternalOutput").ap()
    ctx_d = nc.dram_tensor("ctx", [NB, H], f32, kind="ExternalOutput").ap()

    with tile.TileContext(nc) as tc, ExitStack() as ctx:
        singles = ctx.enter_context(tc.tile_pool(name="singles", bufs=1))
        annT_pool = ctx.enter_context(tc.tile_pool(name="annTp", bufs=6))
        nat_pool = ctx.enter_context(tc.tile_pool(name="natp", bufs=8))
        tanh_pool = ctx.enter_context(tc.tile_pool(name="tanhp", bufs=20))
        exp_pool = ctx.enter_context(tc.tile_pool(name="expp", bufs=2))
        pa_ps = ctx.enter_context(tc.tile_pool(name="paps", bufs=2, space="PSUM"))
        sc_ps = ctx.enter_context(tc.tile_pool(name="scps", bufs=3, space="PSUM"))

        # ---- load weights / small inputs ----
        WaT = []
        for k in range(KH):
            t = singles.tile([128, H], bf, tag=f"WaT{k}")
            nc.sync.dma_start(out=t, in_=WaT_d[k])
            WaT.append(t)
        v_sb = singles.tile([128, KH * 32], bf, tag="vP")
        nc.sync.dma_start(out=v_sb, in_=v_d)
        xTa, WihTa, rsT, WhhT, WqT, bqa = [], [], [], [], [], []
        for k in range(5):
            t = singles.tile([128, NB], f32, tag=f"xTa{k}")
            nc.sync.dma_start(out=t, in_=xTa_d[k])
            xTa.append(t)
            t = singles.tile([128, 3 * H], f32, tag=f"WihTa{k}")
            nc.sync.dma_start(out=t, in_=WihTa_d[k])
            WihTa.append(t)
        for k in range(KH + 1):
            t = singles.tile([128, NB], f32, tag=f"rsT{k}")
            nc.sync.dma_start(out=t, in_=rsT_d[k])
            rsT.append(t)
            t = singles.tile([128, 3 * H], f32, tag=f"WhhT{k}")
            nc.sync.dma_start(out=t, in_=WhhT_d[k])
            WhhT.append(t)
        for k in range(KH):
            t = singles.tile([128, H], f32, tag=f"WqT{k}")
            nc.sync.dma_start(out=t, in_=WqT_d[k])
            WqT.append(t)
            t = singles.tile([128, 1], f32, tag=f"bqa{k}")
            nc.sync.dma_start(out=t, in_=bqa_d[k])
            bqa.append(t)
        rsN = singles.tile([NB, H], f32, tag="rsN")
        nc.sync.dma_start(out=rsN, in_=rsN_d)
        ident = singles.tile([128, 128], f32, tag="ident")
        make_identity(nc, ident)

        # ---- GRU cell ----
        # r/z parts: accumulate x@Wih and h@Whh into ONE psum group
        grz = sc_ps.tile([NB, 512], f32, tag="sc")
        nmm = 5 + KH + 1
        for k in range(5):
            nc.tensor.matmul(grz, lhsT=xTa[k], rhs=WihTa[k][:, 0:512],
                             start=(k == 0), stop=False)
        for k in range(KH + 1):
            nc.tensor.matmul(grz, lhsT=rsT[k], rhs=WhhT[k][:, 0:512],
                             start=False, stop=(k == KH))
        # n parts stay separate (r gates h_n before the add)
        gin = sc_ps.tile([NB, H], f32, tag="sc")
        for k in range(5):
            nc.tensor.matmul(gin, lhsT=xTa[k], rhs=WihTa[k][:, 512:768],
                             start=(k == 0), stop=(k == 4))
        ghn = sc_ps.tile([NB, H], f32, tag="sc")
        for k in range(KH + 1):
            nc.tensor.matmul(ghn, lhsT=rsT[k], rhs=WhhT[k][:, 512:768],
                             start=(k == 0), stop=(k == KH))

        rz = singles.tile([NB, 512], f32, tag="rz")
        nc.scalar.activation(rz, grz, AF.Sigmoid)
        tn = singles.tile([NB, H], f32, tag="tn")
        nc.vector.tensor_mul(tn, rz[:, 0:H], ghn)
        sn = singles.tile([NB, H], f32, tag="sn")
        nc.vector.tensor_add(sn, tn, gin)
        n_sb = singles.tile([NB, H], f32, tag="n_sb")
        nc.scalar.activation(n_sb, sn, AF.Tanh)
        d_sb = singles.tile([NB, H], f32, tag="d_sb")
        nc.vector.tensor_sub(d_sb, rsN, n_sb)
        zd = singles.tile([NB, H], f32, tag="zd")
        nc.vector.tensor_mul(zd, rz[:, H:2 * H], d_sb)
        rnn_sb = singles.tile([NB, H], f32, tag="rnn_sb")
        nc.vector.tensor_add(rnn_sb, n_sb, zd)
        nc.gpsimd.dma_start(out=rnn_d, in_=rnn_sb)

        # transpose rnn_out -> [H, NB] (bf16) for the pq matmul
        rnnT = []
        for k in range(KH):
            ps = sc_ps.tile([128, NB], f32, tag="sc")
            nc.tensor.transpose(ps, rnn_sb[:, k * 128:(k + 1) * 128], ident[:NB, :NB])
            t = singles.tile([128, NB], f32, tag=f"rnnT{k}")
            nc.vector.tensor_copy(t, ps)
            rnnT.append(t)

        # pq.T [h_out, b] + (bq + ba) bias -> tanh bias terms
        pqb = []
        for m in range(KH):
            ps = sc_ps.tile([128, NB], f32, tag="sc")
            for k in range(KH):
                nc.tensor.matmul(ps, lhsT=WqT[k][:, m * 128:(m + 1) * 128],
                                 rhs=rnnT[k], start=(k == 0), stop=(k == KH - 1))
            t = singles.tile([128, NB], f32, tag=f"pqb{m}")
            nc.vector.tensor_scalar_add(t, ps, bqa[m])
            pqb.append(t)

        # ---- attention main loop ----
        ctx_rows = []  # (group sbuf tile) per g
        nat_sb = {}
        for g in range(NG):
            tanh_tiles = {}
            for lb in range(GROUP):
                b = g * GROUP + lb
                aT = []
                for k in range(KH):
                    t = annT_pool.tile([128, T], bf, tag="annT")
                    nc.sync.dma_start(out=t, in_=annT[b * KH + k])
                    aT.append(t)
                t = nat_pool.tile([128, NTC, H], bf, tag="nat")
                nc.sync.dma_start(out=t, in_=annN[b])
                nat_sb[b] = t

                # pa = ann @ Wa.T  (output transposed: [h_out, tokens])
                for m in range(KH):
                    for th in range(NTH):
                        pps = pa_ps.tile([128, PA_W], f32, tag="pa")
                        for q in range(NQ):
                            for k in range(KH):
                                nc.tensor.matmul(
                                    pps[:, q * 512:(q + 1) * 512],
                                    lhsT=WaT[k][:, m * 128:(m + 1) * 128],
                                    rhs=aT[k][:, th * PA_W + q * 512: th * PA_W + (q + 1) * 512],
                                    start=(k == 0), stop=(k == KH - 1))
                        tt = tanh_pool.tile([128, PA_W], bf, tag="tanh")
                        nc.scalar.activation(tt, pps, AF.Tanh, bias=pqb[m][:, b:b + 1])
                        tanh_tiles[(lb, m, th)] = tt

            # scores = tanh @ v for the 4 batches of this group, col-tiled so
            # batch lb lands on psum partition 32*lb
            exp_sb = exp_pool.tile([128, T], f32, tag="exp")
            sums = exp_pool.tile([128, NS], f32, tag="sums")
            for s in range(NS):
                scp = sc_ps.tile([128, 512], f32, tag="sc")
                for lb in range(GROUP):
                    for m in range(KH):
                        nc.tensor.matmul(
                            scp[32 * lb:32 * lb + 32, :],
                            lhsT=v_sb[:, m * 32:(m + 1) * 32],
                            rhs=tanh_tiles[(lb, m, s // NQ)][:, (s % NQ) * 512:(s % NQ + 1) * 512],
                            start=(m == 0), stop=(m == KH - 1),
                            tile_position=(0, 32 * lb))
                nc.scalar.activation(exp_sb[:, s * 512:(s + 1) * 512], scp, AF.Exp,
                                     accum_out=sums[:, s:s + 1])

            rsum = exp_pool.tile([128, 1], f32, tag="rsum")
            nc.vector.reduce_sum(rsum, sums, axis=AX.X)
            rinv = exp_pool.tile([128, 1], f32, tag="rinv")
            nc.vector.reciprocal(rinv, rsum)

            # transpose (unnormalized) exp -> [tokens, batch-col] bf16 for ctx
            eT = exp_pool.tile([128, NTC, 128], bf, tag="expT")
            for tcb in range(NTC):
                tps = sc_ps.tile([128, 128], f32, tag="sc")
                nc.tensor.transpose(tps, exp_sb[:, tcb * 128:(tcb + 1) * 128], ident)
                nc.vector.tensor_copy(eT[:, tcb, :], tps)

            # normalize in place (after the transposes have read exp_sb)
            nc.vector.tensor_scalar_mul(exp_sb, exp_sb, rinv)
            nc.gpsimd.dma_start(out=align_d[g * GROUP:(g + 1) * GROUP],
                                in_=exp_sb[0:128:32, :])

            # ctx[b] = sum_t exp[t] * ann[b, t, :]  (normalized on evac)
            cps = sc_ps.tile([128, H], f32, tag="sc")
            for lb in range(GROUP):
                b = g * GROUP + lb
                for tcb in range(NTC):
                    nc.tensor.matmul(cps[32 * lb:32 * lb + 32, :],
                                     lhsT=eT[:, tcb, 32 * lb:32 * lb + 32],
                                     rhs=nat_sb[b][:, tcb, :],
                                     start=(tcb == 0), stop=(tcb == NTC - 1),
                                     tile_position=(0, 32 * lb))
            cg = exp_pool.tile([128, H], f32, tag="ctxg")
            nc.vector.tensor_scalar_mul(cg, cps, rinv)
            nc.gpsimd.dma_start(out=ctx_d[g * GROUP:(g + 1) * GROUP],
                                in_=cg[0:128:32, :])
            ctx_rows.append(cg)

    nc.compile()
    return nc


def _get_program(T):
    if T not in _PROGRAM_CACHE:
        _PROGRAM_CACHE[T] = _build_program(T)
    return _PROGRAM_CACHE[T]


def _stage_inputs(memory, context, rnn_state, annotations,
                  W_ih, W_hh, b_ih, b_hh, Wq, bq, Wa, ba, v):
    """Build the per-core in_maps (host-side sharding + layout staging)."""
    T = annotations.shape[1]
    NTC = T // 128
    f32 = np.float32

    x = np.concatenate([memory, context], axis=-1).astype(f32)       # [B, 2H]
    xT = np.zeros((5, 128, B), f32)
    xT[:4] = x.T.reshape(4, 128, B)
    xT[4, 0, :] = 1.0                                                # ones row
    WihTa = np.zeros((5, 128, 3 * H), f32)
    WihTa[:4] = W_ih.T.reshape(4, 128, 3 * H)
    WihTa[4, 0, :] = b_ih                                            # bias row
    rsT = np.zeros((KH + 1, 128, B), f32)
    rsT[:KH] = rnn_state.T.reshape(KH, 128, B)
    rsT[KH, 0, :] = 1.0                                              # ones row
    WhhT = np.zeros((KH + 1, 128, 3 * H), f32)
    WhhT[:KH] = W_hh.T.reshape(KH, 128, 3 * H)
    WhhT[KH, 0, :] = b_hh                                            # bias row
    WqT = Wq.T.reshape(KH, 128, H)
    bqa = (bq + ba).reshape(KH, 128, 1).astype(f32)
    WaT = Wa.T.reshape(KH, 128, H)
    vP = np.zeros((128, KH * 32), f32)                               # M=32 pad
    for k in range(KH):
        vP[:, k * 32] = v[k * 128:(k + 1) * 128]

    in_maps = []
    for c in range(NCORES):
        rows = slice(c * NB, (c + 1) * NB)
        ann_c = annotations[rows]                                    # [NB, T, H]
        annT_c = np.ascontiguousarray(ann_c.transpose(0, 2, 1)).astype(BF16)
        annN_c = np.ascontiguousarray(
            ann_c.reshape(NB, NTC, 128, H).transpose(0, 2, 1, 3)).astype(BF16)
        in_maps.append({
            "annT": annT_c.reshape(NB * KH, 128, T),
            "annN": annN_c,
            "WaT": WaT.astype(BF16),
            "vP": vP.astype(BF16),
            "xTa": np.ascontiguousarray(xT[:, :, rows]),
            "WihTa": WihTa,
            "rsT": np.ascontiguousarray(rsT[:, :, rows]),
            "WhhT": np.ascontiguousarray(WhhT.astype(np.float32)),
            "WqT": np.ascontiguousarray(WqT.astype(np.float32)),
            "bqa": bqa,
            "rsN": rnn_state[rows].astype(f32),
        })
    return in_maps


def _run(in_maps, T, trace=False):
    from concourse.bass_utils import run_bass_kernel_spmd
    nc = _get_program(T)
    res = run_bass_kernel_spmd(nc, in_maps, list(range(NCORES)), trace=trace)
    return res


def kernel(**inputs):
    inputs = {k: np.asarray(v) for k, v in inputs.items()}
    T = inputs["annotations"].shape[1]
    in_maps = _stage_inputs(**inputs)
    res = _run(in_maps, T)
    outs = res.results
    rnn_output = np.concatenate([outs[c]["rnn_out"] for c in range(NCORES)], 0)
    new_context = np.concatenate([outs[c]["ctx"] for c in range(NCORES)], 0)
    alignment = np.concatenate([outs[c]["align"] for c in range(NCORES)], 0)
    return (rnn_output.astype(np.float32), new_context.astype(np.float32),
            alignment.astype(np.float32))
